# revision 7
# baseline (speedup 1.0000x reference)
"""Trainium2 Bass kernel for an RK4 neural-ODE solver (DiffeqSolver).

Math (matches the jax reference):
    f(y) = tanh(tanh(y@W1+b1)@W2+b2)@W3 + b3
    RK4 over T-1 intervals of time grid t; output is the trajectory.

Strategy:
  - Data-parallel over the S*B = 2048 trajectories: 256 rows per core,
    8 cores, weights replicated. No collectives.
  - Everything on-chip lives transposed as [feature, batch] so the
    weight matrices are the stationary matmul operands in their natural
    [K, M] layout and no transposes are needed in the time loop.
  - Matmul inputs in bf16 (fp32 PSUM accumulate); state, RK4 combines
    and outputs in fp32.  Validated: ~1.4e-3 scale-relative absmax
    error vs the fp32 reference.
  - tanh is done on the scalar engine straight out of PSUM over merged
    [128, 512] tiles (two adjacent matmul output tiles per activation
    instruction) to amortize the ~352-cycle ACT overhead.
  - RK4 combines use fused DVE scalar_tensor_tensor:
        ymid = (psum * c) + y   (bf16 out, feeds next matmul)
        acc  = (psum * c) + acc (fp32 running y_next)
    with the dt-derived scales baked as immediates (the program is
    compiled per call from the actual inputs).
"""

import numpy as np
import ml_dtypes

S, B, D, H, T = 4, 512, 256, 1024, 50
NCORES = 8
NSH = S * B // NCORES  # 256 rows per core
NSTEPS = T - 1

F32 = None  # filled lazily (mybir import is heavy; keep module import light)


def _build_program(dts: np.ndarray, b1: np.ndarray, b2: np.ndarray, b3: np.ndarray,
                   n_steps: int):
    """Builds the Bass program. dts: fp32[n_steps] interval widths."""
    import concourse.bass as bass
    import concourse.mybir as mybir
    import concourse.tile as tile
    from concourse import bacc

    f32 = mybir.dt.float32
    bf16 = mybir.dt.bfloat16
    AF = mybir.ActivationFunctionType
    ALU = mybir.AluOpType

    nc = bacc.Bacc()

    # DRAM I/O.  Host pre-layouts everything:
    #  y0   [128, 2*NSH]  fp32 : col kd*NSH+n, part p  <-> y0[n, kd*128+p]
    #  w1   [128, 2*H]    bf16 : col kd*H + m*128+j    <-> W1[kd*128+p, m*128+j]
    #  w2   [128, 8*H]    bf16 : col k*H + m*128+j     <-> W2[k*128+p, m*128+j]
    #  w3   [128, 8*D]    bf16 : col k*D + d*128+j     <-> W3[k*128+p, d*128+j]
    #  yout [n_steps, 128, 2*NSH] fp32, same layout as y0 per step
    y0_d = nc.dram_tensor("y0", [128, 2 * NSH], f32, kind="ExternalInput")
    w1_d = nc.dram_tensor("w1", [128, 2 * H], bf16, kind="ExternalInput")
    w2_d = nc.dram_tensor("w2", [128, 8 * H], bf16, kind="ExternalInput")
    w3_d = nc.dram_tensor("w3", [128, 8 * D], bf16, kind="ExternalInput")
    yout_d = nc.dram_tensor("yout", [n_steps, 128, 2 * NSH], f32,
                            kind="ExternalOutput")

    zero_bias = not (np.any(b1) or np.any(b2) or np.any(b3))

    from contextlib import ExitStack
    with tile.TileContext(nc) as tc, ExitStack() as es:
        consts = es.enter_context(tc.tile_pool(name="consts", bufs=1))
        ypool = es.enter_context(tc.tile_pool(name="ypool", bufs=2))
        ybfpool = es.enter_context(tc.tile_pool(name="ybfpool", bufs=2))
        accpool = es.enter_context(tc.tile_pool(name="accpool", bufs=2))
        hpool = es.enter_context(tc.tile_pool(name="hpool", bufs=2))
        psh = es.enter_context(tc.tile_pool(name="psh", bufs=6, space="PSUM"))
        psk = es.enter_context(tc.tile_pool(name="psk", bufs=2, space="PSUM"))

        w1_sb = consts.tile([128, 2 * H], bf16)
        w2_sb = consts.tile([128, 8 * H], bf16)
        w3_sb = consts.tile([128, 8 * D], bf16)
        nc.sync.dma_start(w1_sb[:], w1_d[:])
        nc.sync.dma_start(w2_sb[:], w2_d[:])
        nc.sync.dma_start(w3_sb[:], w3_d[:])

        if not zero_bias:
            # per-partition bias tables: col m holds b[m*128:(m+1)*128]
            b1_sb = consts.tile([128, 8], f32)
            b2_sb = consts.tile([128, 8], f32)
            b3_sb = consts.tile([128, 2], f32)
            b1_d = nc.dram_tensor("b1t", [128, 8], f32, kind="ExternalInput")
            b2_d = nc.dram_tensor("b2t", [128, 8], f32, kind="ExternalInput")
            b3_d = nc.dram_tensor("b3t", [128, 2], f32, kind="ExternalInput")
            nc.sync.dma_start(b1_sb[:], b1_d[:])
            nc.sync.dma_start(b2_sb[:], b2_d[:])
            nc.sync.dma_start(b3_sb[:], b3_d[:])

        y_cur = ypool.tile([128, 2 * NSH], f32, tag="y")
        nc.sync.dma_start(y_cur[:], y0_d[:])
        ybf = ybfpool.tile([128, 2 * NSH], bf16, tag="ybf")
        nc.vector.tensor_copy(ybf[:], y_cur[:])

        def mlp_eval(ybf_in):
            """Emits one f() evaluation; returns the [128, 512] psum pair
            holding k~ = (mm3 output, no b3) with d-tile 0 in ps3[0] and
            d-tile 1 in ps3[1]."""
            # ---- layer 1: h1 = tanh(W1.T @ ymid.T + b1) ----
            # m-outer, kd-inner: PSUM accumulation groups MUST be contiguous
            # on the PE queue (interleaving start/stop groups corrupts the
            # accumulation and can wedge the device).
            ps1 = [psh.tile([128, 2 * NSH], f32, tag="psh", name=f"ps1_{g}") for g in range(4)]
            for m in range(8):
                g, mloc = divmod(m, 2)
                for kd in range(2):
                    nc.tensor.matmul(
                        ps1[g][:, mloc * NSH:(mloc + 1) * NSH],
                        w1_sb[:, kd * H + m * 128: kd * H + (m + 1) * 128],
                        ybf_in[:, kd * NSH:(kd + 1) * NSH],
                        start=(kd == 0), stop=(kd == 1))
            h1 = hpool.tile([128, 8 * NSH], bf16, tag="h1")
            for g in range(4):
                if zero_bias:
                    nc.scalar.activation(h1[:, g * 512:(g + 1) * 512],
                                         ps1[g][:], AF.Tanh)
                else:
                    for mloc in range(2):
                        m = 2 * g + mloc
                        nc.scalar.activation(
                            h1[:, m * NSH:(m + 1) * NSH],
                            ps1[g][:, mloc * NSH:(mloc + 1) * NSH],
                            AF.Tanh, bias=b1_sb[:, m:m + 1])
            # ---- layer 2: h2 = tanh(W2.T @ h1 + b2) ----
            ps2 = [psh.tile([128, 2 * NSH], f32, tag="psh", name=f"ps2_{g}") for g in range(4)]
            for m2 in range(8):  # m-outer: psum groups finish early for tanh2
                g, mloc = divmod(m2, 2)
                for k in range(8):
                    nc.tensor.matmul(
                        ps2[g][:, mloc * NSH:(mloc + 1) * NSH],
                        w2_sb[:, k * H + m2 * 128: k * H + (m2 + 1) * 128],
                        h1[:, k * NSH:(k + 1) * NSH],
                        start=(k == 0), stop=(k == 7))
            h2 = hpool.tile([128, 8 * NSH], bf16, tag="h2")
            for g in range(4):
                if zero_bias:
                    nc.scalar.activation(h2[:, g * 512:(g + 1) * 512],
                                         ps2[g][:], AF.Tanh)
                else:
                    for mloc in range(2):
                        m = 2 * g + mloc
                        nc.scalar.activation(
                            h2[:, m * NSH:(m + 1) * NSH],
                            ps2[g][:, mloc * NSH:(mloc + 1) * NSH],
                            AF.Tanh, bias=b2_sb[:, m:m + 1])
            # ---- layer 3: k~ = W3.T @ h2 (b3 folded into the combines) ----
            ps3 = [psk.tile([128, NSH], f32, tag="psk", name=f"ps3_{d}") for d in range(2)]
            for d in range(2):  # d-outer: d-tile 0 ready early for ymid
                for k in range(8):
                    nc.tensor.matmul(
                        ps3[d][:],
                        w3_sb[:, k * D + d * 128: k * D + (d + 1) * 128],
                        h2[:, k * NSH:(k + 1) * NSH],
                        start=(k == 0), stop=(k == 7))
            return ps3

        for i in range(n_steps):
            dt = float(dts[i])
            c_mid = [0.5 * dt, 0.5 * dt, dt]           # ymid coefficients
            c_acc = [dt / 6.0, dt / 3.0, dt / 3.0, dt / 6.0]
            acc = y_cur
            ybf_next = None
            for e in range(4):
                ps3 = mlp_eval(ybf)
                if not zero_bias:
                    # materialize k = k~ + b3 in SBUF, use it as the source
                    kfull = accpool.tile([128, 2 * NSH], f32, tag="kfull")
                    for d in range(2):
                        nc.vector.tensor_scalar_add(
                            kfull[:, d * NSH:(d + 1) * NSH], ps3[d][:],
                            b3_sb[:, d:d + 1])
                    ksrc = [kfull[:, 0:NSH], kfull[:, NSH:2 * NSH]]
                else:
                    ksrc = [ps3[0][:], ps3[1][:]]
                # ymid for the next eval (critical path -> emit first,
                # per d-tile so d-tile 0 unblocks mm1's kd=0 pass asap)
                if e < 3:
                    ybf_next = ybfpool.tile([128, 2 * NSH], bf16, tag="ybf", name="ybf_next")
                    for d in range(2):
                        sl = slice(d * NSH, (d + 1) * NSH)
                        nc.vector.scalar_tensor_tensor(
                            ybf_next[:, sl], ksrc[d], c_mid[e], y_cur[:, sl],
                            op0=ALU.mult, op1=ALU.add)
                # acc += c_acc * k   (fp32; off critical path)
                new_acc = (accpool.tile([128, 2 * NSH], f32, tag="acc", name="accf")
                           if e == 3 else
                           accpool.tile([128, 2 * NSH], f32, tag="acctmp", name="acct"))
                for d in range(2):
                    sl = slice(d * NSH, (d + 1) * NSH)
                    nc.vector.scalar_tensor_tensor(
                        new_acc[:, sl], ksrc[d], c_acc[e], acc[:, sl],
                        op0=ALU.mult, op1=ALU.add)
                acc = new_acc
                if e < 3:
                    ybf = ybf_next
            y_cur = acc
            nc.sync.dma_start(yout_d[i], y_cur[:])
            if i + 1 < n_steps:
                ybf = ybfpool.tile([128, 2 * NSH], bf16, tag="ybf")
                nc.vector.tensor_copy(ybf[:], y_cur[:])

    nc.finalize()
    return nc


def _f32imm(x):
    return float(np.float32(x))


def _host_layout(inputs):
    """Prepares per-core in_maps (host-side shard + transpose + cast)."""
    fp = np.asarray(inputs["first_point"], dtype=np.float32).reshape(S * B, D)
    t = np.asarray(inputs["time_steps_to_predict"], dtype=np.float32)
    dts = (t[1:] - t[:-1]).astype(np.float32)
    W1 = np.asarray(inputs["W1"], dtype=np.float32)
    W2 = np.asarray(inputs["W2"], dtype=np.float32)
    W3 = np.asarray(inputs["W3"], dtype=np.float32)
    b1 = np.asarray(inputs["b1"], dtype=np.float32)
    b2 = np.asarray(inputs["b2"], dtype=np.float32)
    b3 = np.asarray(inputs["b3"], dtype=np.float32)

    def wlay(W):  # [K, M] -> [128, (K//128)*M], col k*M + j
        Kd, M = W.shape
        blocks = [W[k * 128:(k + 1) * 128, :] for k in range(Kd // 128)]
        return np.ascontiguousarray(
            np.concatenate(blocks, axis=1).astype(ml_dtypes.bfloat16))

    w1h, w2h, w3h = wlay(W1), wlay(W2), wlay(W3)

    zero_bias = not (np.any(b1) or np.any(b2) or np.any(b3))
    bias_maps = {}
    if not zero_bias:
        bias_maps = {
            "b1t": np.ascontiguousarray(b1.reshape(8, 128).T.astype(np.float32)),
            "b2t": np.ascontiguousarray(b2.reshape(8, 128).T.astype(np.float32)),
            "b3t": np.ascontiguousarray(b3.reshape(2, 128).T.astype(np.float32)),
        }

    in_maps = []
    for c in range(NCORES):
        shard = fp[c * NSH:(c + 1) * NSH, :]          # [NSH, D]
        sT = shard.T                                   # [D, NSH]
        y0 = np.concatenate([sT[0:128, :], sT[128:256, :]], axis=1)
        in_maps.append({
            "y0": np.ascontiguousarray(y0.astype(np.float32)),
            "w1": w1h, "w2": w2h, "w3": w3h, **bias_maps,
        })
    return in_maps, dts, (b1, b2, b3)


def _assemble(results, n_steps, first_point):
    """results[c]["yout"]: [n_steps, 128, 2*NSH] -> pred_y [S,B,T,D]."""
    shards = []
    for c in range(NCORES):
        arr = np.asarray(results[c]["yout"])            # [n, 128, 512]
        arr = arr.reshape(n_steps, 128, 2, NSH)          # [n, p, kd, nsh]
        arr = arr.transpose(3, 0, 2, 1).reshape(NSH, n_steps, D)  # [nsh, n, d]
        shards.append(arr)
    traj = np.concatenate(shards, axis=0)                # [S*B, n_steps, D]
    fp = np.asarray(first_point, dtype=np.float32).reshape(S * B, 1, D)
    full = np.concatenate([fp, traj], axis=1)            # [S*B, T, D]
    return np.ascontiguousarray(
        full.reshape(S, B, n_steps + 1, D)).astype(np.float32)


def run(inputs, n_steps=NSTEPS, **spmd_kwargs):
    """Builds, runs on 8 cores, returns (pred_y, reg_state, raw_results)."""
    from concourse.bass_utils import run_bass_kernel_spmd

    in_maps, dts, (b1, b2, b3) = _host_layout(inputs)
    nc = _build_program(dts, b1, b2, b3, n_steps)
    res = run_bass_kernel_spmd(nc, in_maps, core_ids=list(range(NCORES)),
                               **spmd_kwargs)
    pred_y = _assemble(res.results, n_steps, inputs["first_point"])
    reg_state = np.zeros((S,), dtype=np.float32)
    return pred_y, reg_state, res


def kernel(**inputs):
    pred_y, reg_state, _ = run(inputs)
    return pred_y, reg_state
